# revision 22
# baseline (speedup 1.0000x reference)
"""Multi-head attention (B=4, S=2048, D=1024, H=16) + output projection on 8 trn2 cores.

Sharding: no collectives. Core c handles batch c//2, query rows (c%2)*1024..+1024,
all 16 heads. Each core needs full K/V for its batch; W_out/b_out replicated.
The per-core output block [1024, 1024] is the final projected output for those
query rows, so the host just concatenates.

Per-core pipeline (all matmuls bf16, fp32 PSUM accumulation):
  - softmax exp is split across TWO engines: ScalarE exact Exp activation on
    ~20/32 tiles per head pair, DVE on the other 12 via a Schraudolph-style
    int16 trick: bf16_bits(exp(s*SCALE)) ~= int16(s*EXPA + EXPB), one
    tensor_scalar (mult,add) op with an int16-bitcast view of the bf16 expt
    tile.  (DVE fp32->int16 convert truncates; EXPB folds in the +0.5 and the
    Schraudolph centering constant.)
  - psum: 2 score slots [128,1024] + 2 AV accumulators [65,1024] = 8 banks.
    Issue order is software-pipelined: scores(jc+1) is emitted BEFORE AV(jc),
    so the steady-state chain is exp(jc) -> scores(jc+1) (~1.6us) rather than
    exp -> AV -> scores.  Score matmuls are emitted h0,h64 adjacent so the
    row-disjoint pairs execute concurrently in the PE array.
  - AV lhsT = v_aug [128 j, 65] (ones column -> softmax sums for free),
    accumulated over jc with start/stop into av[h2] psum.
  - normalize: DVE copy av->SBUF (releases psum), fast reciprocal of the sums
    row, gpsimd partition_broadcast, DVE multiply -> attT bf16.  The ops are
    spread over the next head pair's jc loop so they never head-of-line block
    the DVE FIFO at a pair boundary.
  - DMA: only SWDGE (gpsimd queue) can cast fp32->bf16, so q/k/v/w casts all
    live there, emitted in need-order (w in 8 column chunks spread over
    hp=2..5; v per-head chunks 2 pairs ahead).  Transposed loads (d on
    partitions) via HWDGE DMA-transpose on the sync queue; k chunk-0/1
    transposes go on the scalar HWDGE queue so the first scores matmul is
    gated by max(q0,k0) instead of their sum.
  - projection: final[i, e] = attT.T @ WT accumulated over the 8 d-chunks in
    psum, bias added on DVE from a partition-broadcast bias tile, fp32 out.
"""

import numpy as np

import concourse.bass as bass
import concourse.tile as tile
from concourse import bacc, mybir
from concourse.bass_utils import run_bass_kernel_spmd

B = 4
S = 2048
DM = 1024
H = 16
DK = 64
SCALE = DK**-0.5
I = 1024  # local query rows per core
NJC = S // 128  # 16 j-chunks
NHP = H // 2  # 8 head pairs == 8 d-chunks of the model dim

F32 = mybir.dt.float32
BF16 = mybir.dt.bfloat16
I16 = mybir.dt.int16

# Schraudolph exp constants: int16(s*EXPA + EXPB) viewed as bf16 bits is
# exp(s*SCALE) to within ~+-3.2%.  EXPA = SCALE*log2(e)*128; EXPB = 16256
# (=127<<7) - 7.5 (spread centering); HW convert rounds-to-nearest (CoreSim
# truncates — sim error is slightly off-center, HW is what matters).
EXPA = SCALE * np.log2(np.e) * 128.0
EXPB = 16248.5

N_WARMUP = 26  # bridge the DMA prelude so HAM never re-throttles


def build(nc: bass.Bass):
    q = nc.dram_tensor("q", [I, DM], F32, kind="ExternalInput").ap()
    k = nc.dram_tensor("k", [S, DM], F32, kind="ExternalInput").ap()
    v = nc.dram_tensor("v", [S, DM], F32, kind="ExternalInput").ap()
    w = nc.dram_tensor("w", [DM, DM], F32, kind="ExternalInput").ap()
    b = nc.dram_tensor("b", [DM], F32, kind="ExternalInput").ap()
    out = nc.dram_tensor("out", [I, DM], F32, kind="ExternalOutput").ap()

    # bf16 DRAM staging for q/k column chunks 1-2 (chunk 0 goes through an
    # on-chip HWDGE-load + DVE-cast + block-transpose path that avoids the
    # SWDGE completion-latency on the critical prelude path).
    CSPLIT = [(0, 128), (128, 512), (512, 1024)]
    q_bf = [None] + [
        nc.dram_tensor(f"q_bf{i}", [I, c1 - c0], BF16).ap()
        for i, (c0, c1) in list(enumerate(CSPLIT))[1:]
    ]
    k_bf = [None] + [
        nc.dram_tensor(f"k_bf{i}", [S, c1 - c0], BF16).ap()
        for i, (c0, c1) in list(enumerate(CSPLIT))[1:]
    ]
    w_bf = nc.dram_tensor("w_bf", [DM, DM], BF16).ap()
    # v bounced through DRAM as bf16 (2 big SWDGE casts), then filled into
    # SBUF per-head by non-cast HWDGE loads on the sync queue — keeps the
    # gpsimd DMA stream down to 9 descriptors so the normalize broadcasts
    # are never starved behind semaphore-recycle fences.
    v_bfd = [
        nc.dram_tensor("v_bfd0", [S, 256], BF16).ap(),
        nc.dram_tensor("v_bfd1", [S, 768], BF16).ap(),
    ]

    with tile.TileContext(nc) as tc:
        with (
            tc.tile_pool(name="persist", bufs=1) as pers,
            tc.tile_pool(name="expp", bufs=4) as expp,
            tc.tile_pool(name="avsbp", bufs=1) as avsbp,
            tc.tile_pool(name="nrmp", bufs=1) as nrmp,
            tc.tile_pool(name="finp", bufs=2) as finp,
        ):
            warm_sb = pers.tile([128, 512], BF16, name="warm_sb", tag="warm_sb")
            nc.vector.memset(warm_sb[:, :], 0.0)

            # ---- v_aug: all 16 heads side by side, ones column at 64 ----
            vA_all = pers.tile(
                [128, H * NJC * 65], BF16, name="vA_all", tag="vA_all"
            )
            vA4 = vA_all[:, :].rearrange("p (h jc e) -> p h jc e", h=H, e=65)
            nc.vector.memset(vA4[:, :, :, DK], 1.0)

            def load_v_h(h):
                # non-cast HWDGE load from the bf16 bounce buffer (sync queue)
                if h < 4:
                    src = v_bfd[0][:, h * DK : (h + 1) * DK]
                else:
                    src = v_bfd[1][:, (h - 4) * DK : (h - 3) * DK]
                nc.sync.dma_start(
                    out=vA4[:, h, :, 0:DK],
                    in_=src.rearrange("(jc p) d -> p jc d", p=128),
                )

            def vA(h):
                return vA_all[:, h * NJC * 65 : (h + 1) * NJC * 65]

            # ---- SWDGE (gpsimd) cast queue: 7 DMAs total, in need-order ----
            nc.gpsimd.dma_start(out=q_bf[1][:, :], in_=q[:, 128:512])
            nc.gpsimd.dma_start(out=k_bf[1][:, :], in_=k[:, 128:512])
            nc.gpsimd.dma_start(out=v_bfd[0][:, :], in_=v[:, 0:256])

            def gp_prefetch(hp):
                if hp == 0:
                    nc.gpsimd.dma_start(out=v_bfd[1][:, :], in_=v[:, 256:1024])
                    nc.gpsimd.dma_start(out=q_bf[2][:, :], in_=q[:, 512:1024])
                    nc.gpsimd.dma_start(out=k_bf[2][:, :], in_=k[:, 512:1024])
                elif hp == 1:
                    nc.gpsimd.dma_start(out=w_bf[:, :], in_=w[:, :])

            # ---- HWDGE transposed loads: qT/kT with head_dim on partitions ----
            qTh, kTh = [], []
            for i, (c0, c1) in enumerate(CSPLIT):
                nb = (c1 - c0) // 128
                qt = pers.tile([128, nb, I], BF16, name=f"qTh{i}", tag=f"qTh{i}")
                qTh.append(qt)
                kt = pers.tile([128, nb, S], BF16, name=f"kTh{i}", tag=f"kTh{i}")
                kTh.append(kt)
            # prelude chunk 0: fp32 HWDGE loads (fast completion), DVE cast,
            # then per-128-row-block SBUF->SBUF DMA transposes.  q on sync,
            # k on scalar queue so the two paths run in parallel.
            q0_sb = pers.tile([128, I // 128, 128], F32, name="q0_sb", tag="q0_sb")
            k0_sb = pers.tile([128, S // 128, 128], F32, name="k0_sb", tag="k0_sb")
            nc.sync.dma_start(
                out=q0_sb[:, :, :],
                in_=q[:, 0:128].rearrange("(rb p) c -> p rb c", p=128),
            )
            nc.scalar.dma_start(
                out=k0_sb[:, :, :],
                in_=k[:, 0:128].rearrange("(rb p) c -> p rb c", p=128),
            )
            q0_bfs = pers.tile([128, I // 128, 128], BF16, name="q0_bfs", tag="q0_bfs")
            k0_bfs = pers.tile([128, S // 128, 128], BF16, name="k0_bfs", tag="k0_bfs")
            nc.vector.tensor_copy(q0_bfs[:, :, :], q0_sb[:, :, :])
            nc.vector.tensor_copy(k0_bfs[:, :, :], k0_sb[:, :, :])
            for rb in range(I // 128):
                nc.sync.dma_start(
                    out=qTh[0][:, 0, rb * 128 : (rb + 1) * 128],
                    in_=q0_bfs[:, rb, :],
                    transpose=True,
                )
            for rb in range(S // 128):
                nc.scalar.dma_start(
                    out=kTh[0][:, 0, rb * 128 : (rb + 1) * 128],
                    in_=k0_bfs[:, rb, :],
                    transpose=True,
                )
            nc.sync.dma_start(out=qTh[1][:, :, :], in_=q_bf[1][:, :], transpose=True)
            nc.scalar.dma_start(out=kTh[1][:, :, :], in_=k_bf[1][:, :], transpose=True)
            for h in range(4):
                load_v_h(h)

            wT_all = pers.tile([128, NHP, DM], BF16, name="wT_all", tag="wT_all")
            bias_sb = pers.tile([1, DM], F32, name="bias_sb", tag="bias_sb")
            nc.sync.dma_start(out=bias_sb[:, :], in_=b[None, :])
            bias_bc = pers.tile([128, DM], F32, name="bias_bc", tag="bias_bc")

            def sp_prefetch(hp):
                if 0 <= hp <= 5:
                    load_v_h(2 * hp + 4)
                    load_v_h(2 * hp + 5)
                if hp == 1:
                    nc.sync.dma_start(
                        out=qTh[2][:, :, :], in_=q_bf[2][:, :], transpose=True
                    )
                    nc.sync.dma_start(
                        out=kTh[2][:, :, :], in_=k_bf[2][:, :], transpose=True
                    )
                elif hp == 3:
                    nc.sync.dma_start(
                        out=wT_all[:, :, :], in_=w_bf[:, :], transpose=True
                    )

            def _chunk(hp):
                return (0, 0) if hp == 0 else ((1, hp - 1) if hp < 4 else (2, hp - 4))

            def qT(hp):
                i, e = _chunk(hp)
                return qTh[i][:, e, :]

            def kT(hp):
                i, e = _chunk(hp)
                return kTh[i][:, e, :]

            def wT(dc):
                return wT_all[:, dc, :]

            attT = []
            for hp in range(NHP):
                at = pers.tile([128, I], BF16, name=f"attT{hp}", tag=f"attT{hp}")
                attT.append(at)

            # PE warmup covering the DMA prelude
            with tc.tile_pool(name="warmp", bufs=1, space="PSUM") as warmp:
                wps = warmp.tile([128, 512], F32, name="wps", tag="wps")
                for _ in range(N_WARMUP):
                    nc.tensor.matmul(
                        wps[:, :],
                        warm_sb[:, 0:128],
                        warm_sb[:, :],
                        start=True,
                        stop=True,
                        skip_group_check=True,
                    )

            # ---- attention ----
            with (
                tc.tile_pool(name="smmp", bufs=1, space="PSUM") as smmp,
                tc.tile_pool(name="avp", bufs=1, space="PSUM") as avp,
            ):
                # deferred emission state
                pend_av = None  # (expt pair, av pair, jc, hp) awaiting AV matmuls
                norm_q = []  # deferred normalize callbacks, one popped per jc

                def _normalize_ops(p_hp, p_av):
                    asb = []
                    sums = []
                    # immediate: copies that release the av psum slots (h1
                    # first — its slot is needed sooner by the next pair).
                    # asb copies on DVE; sums-row copies on ScalarE (spreads
                    # the release work over both engines).
                    # NOTE: reciprocal_approx_fast (custom DVE op) corrupts
                    # data when its input is partition-shifted, so the sums
                    # row is copied to partition 0 by a plain copy first.
                    for h2 in (1, 0):
                        a = avsbp.tile(
                            [DK, I], F32, name=f"avsb{p_hp}_{h2}", tag=f"avsb{h2}"
                        )
                        sm = nrmp.tile(
                            [1, I], F32, name=f"sm{p_hp}_{h2}", tag=f"sm{h2}"
                        )
                        nc.vector.tensor_copy(a[:, :], p_av[h2][0:DK, :])
                        nc.scalar.copy(sm[:, :], p_av[h2][DK : DK + 1, :])
                        asb.append(a)
                        sums.append(sm)
                    asb.reverse()
                    sums.reverse()
                    rr = [None, None]
                    rb = [None, None]

                    def mk_recip(h2):
                        def _op():
                            rr[h2] = nrmp.tile(
                                [1, I], F32, name=f"rc{p_hp}_{h2}", tag=f"rc{h2}"
                            )
                            nc.vector.reciprocal_approx_fast(
                                rr[h2][:, :], sums[h2][:, :]
                            )
                            rb[h2] = nrmp.tile(
                                [DK, I], F32, name=f"rb{p_hp}_{h2}", tag=f"rb{h2}"
                            )
                            nc.gpsimd.partition_broadcast(
                                rb[h2][:, :], rr[h2][0:1, :]
                            )

                        return _op

                    def mk_mult(h2):
                        def _op():
                            nc.vector.tensor_mul(
                                attT[p_hp][h2 * DK : (h2 + 1) * DK, :],
                                asb[h2][0:DK, :],
                                rb[h2][:, :],
                            )

                        return _op

                    return [mk_recip(0), mk_recip(1), mk_mult(0), mk_mult(1)]

                av_tiles = {}
                for hp in range(NHP):
                    gp_prefetch(hp)
                    sp_prefetch(hp)
                    av = [
                        avp.tile([65, I], F32, name=f"av{hp}_{h2}", tag=f"av{h2}")
                        for h2 in range(2)
                    ]
                    av_tiles[hp] = av
                    for jc in range(NJC):
                        # scores into ONE 4-bank psum tile (h2*1024 + ih*512
                        # columns): both h2 exps then run CONCURRENTLY on the
                        # two engines and the slots free together, so the
                        # next jc's h0/h64 score pairs issue adjacent and
                        # overlap in the PE array (disjoint row groups).
                        smm = smmp.tile(
                            [128, 2 * I], F32, name=f"smm{hp}_{jc}", tag="smm"
                        )
                        for ih in range(2):
                            for h2 in range(2):
                                nc.tensor.matmul(
                                    smm[
                                        :,
                                        h2 * I + ih * 512 : h2 * I + (ih + 1) * 512,
                                    ],
                                    kT(hp)[
                                        h2 * DK : (h2 + 1) * DK,
                                        jc * 128 : (jc + 1) * 128,
                                    ],
                                    qT(hp)[
                                        h2 * DK : (h2 + 1) * DK,
                                        ih * 512 : (ih + 1) * 512,
                                    ],
                                    start=True,
                                    stop=True,
                                    tile_position=(h2 * DK, 0),
                                )
                        # exp: h2=0 on ScalarE (exact), h2=1 on DVE (int16
                        # Schraudolph), concurrently
                        expt = [
                            expp.tile(
                                [128, I], BF16, name=f"ex{hp}_{jc}_{h2}", tag="expt"
                            )
                            for h2 in range(2)
                        ]
                        nc.scalar.activation(
                            expt[0][:, :],
                            smm[:, 0:I],
                            mybir.ActivationFunctionType.Exp,
                            scale=SCALE,
                        )
                        nc.vector.tensor_scalar(
                            expt[1][:, :].bitcast(I16),
                            smm[:, I : 2 * I],
                            EXPA,
                            EXPB,
                            mybir.AluOpType.mult,
                            mybir.AluOpType.add,
                        )
                        # AV of the previous jc (software pipelining)
                        if pend_av is not None:
                            p_expt, p_av, p_jc, p_hp = pend_av
                            for h2 in (1, 0):
                                for ih in range(2):
                                    nc.tensor.matmul(
                                        p_av[h2][:, ih * 512 : (ih + 1) * 512],
                                        vA(2 * p_hp + h2)[
                                            :, p_jc * 65 : p_jc * 65 + 65
                                        ],
                                        p_expt[h2][:, ih * 512 : (ih + 1) * 512],
                                        start=(p_jc == 0),
                                        stop=(p_jc == NJC - 1),
                                        skip_group_check=True,
                                    )
                            if p_jc == NJC - 1:
                                # pair p_hp done: emit psum-releasing copies now
                                # (h1 first: its slot is needed sooner), defer
                                # the rest across upcoming jc's
                                norm_q.extend(_normalize_ops(p_hp, av_tiles[p_hp]))
                        pend_av = (expt, av, jc, hp)
                        # pop at most one deferred normalize op per jc
                        if norm_q:
                            norm_q.pop(0)()

                # flush: last jc's AV + remaining normalize ops
                p_expt, p_av, p_jc, p_hp = pend_av
                for h2 in (1, 0):
                    for ih in range(2):
                        nc.tensor.matmul(
                            p_av[h2][:, ih * 512 : (ih + 1) * 512],
                            vA(2 * p_hp + h2)[:, p_jc * 65 : p_jc * 65 + 65],
                            p_expt[h2][:, ih * 512 : (ih + 1) * 512],
                            start=(p_jc == 0),
                            stop=(p_jc == NJC - 1),
                            skip_group_check=True,
                        )
                norm_q.extend(_normalize_ops(p_hp, av_tiles[p_hp]))
                for op in norm_q:
                    op()
                norm_q = []

                # keep PE warm across the normalize -> projection handoff
                wps2 = smmp.tile([128, I], F32, name="wps2", tag="smm")
                for _ in range(8):
                    nc.tensor.matmul(
                        wps2[:, 0:512],
                        warm_sb[:, 0:128],
                        warm_sb[:, :],
                        start=True,
                        stop=True,
                        skip_group_check=True,
                    )

            # bias broadcast: end of the gpsimd stream, needed only by proj
            nc.gpsimd.partition_broadcast(bias_bc[:, :], bias_sb[0:1, :])

            # ---- output projection ----
            with tc.tile_pool(name="projp", bufs=4, space="PSUM") as projp:
                for ic in range(I // 128):
                    pp = [
                        projp.tile([128, 512], F32, name=f"pp{ic}_{ec}", tag="pp")
                        for ec in range(2)
                    ]
                    for dc in range(NHP):
                        for ec in range(2):
                            nc.tensor.matmul(
                                pp[ec][:, :],
                                attT[dc][:, ic * 128 : (ic + 1) * 128],
                                wT(dc)[:, ec * 512 : (ec + 1) * 512],
                                start=(dc == 0),
                                stop=(dc == NHP - 1),
                                skip_group_check=True,
                            )
                    fin = finp.tile([128, DM], F32, name=f"fin{ic}", tag="fin")
                    for ec in range(2):
                        nc.vector.tensor_add(
                            fin[:, ec * 512 : (ec + 1) * 512],
                            pp[ec][:, :],
                            bias_bc[:, ec * 512 : (ec + 1) * 512],
                        )
                    nc.sync.dma_start(
                        out=out[ic * 128 : (ic + 1) * 128, :], in_=fin[:, :]
                    )
    return nc


_NC_CACHE = {}


def _get_nc():
    if "nc" not in _NC_CACHE:
        nc = bacc.Bacc("TRN2", target_bir_lowering=False, debug=False)
        build(nc)
        nc.compile()
        _NC_CACHE["nc"] = nc
    return _NC_CACHE["nc"]


def kernel(q, k, v, W_out, b_out, _trace=False, _trace_kwargs=None):
    q = np.asarray(q, dtype=np.float32)
    k = np.asarray(k, dtype=np.float32)
    v = np.asarray(v, dtype=np.float32)
    W_out = np.ascontiguousarray(np.asarray(W_out, dtype=np.float32))
    b_out = np.ascontiguousarray(np.asarray(b_out, dtype=np.float32))

    nc = _get_nc()
    in_maps = []
    for c in range(8):
        bi, half = c // 2, c % 2
        in_maps.append(
            {
                "q": np.ascontiguousarray(q[bi, half * I : (half + 1) * I, :]),
                "k": np.ascontiguousarray(k[bi]),
                "v": np.ascontiguousarray(v[bi]),
                "w": W_out,
                "b": b_out,
            }
        )
    res = run_bass_kernel_spmd(
        nc,
        in_maps,
        core_ids=list(range(8)),
        trace=_trace,
        **(_trace_kwargs or {}),
    )
    out = np.empty((B, S, DM), np.float32)
    for c in range(8):
        bi, half = c // 2, c % 2
        out[bi, half * I : (half + 1) * I, :] = res.results[c]["out"]
    if _trace:
        return out, res
    return out


# revision 27
# speedup vs baseline: 1.1308x; 1.1308x over previous
"""Multi-head attention (B=4, S=2048, D=1024, H=16) + output projection on 8 trn2 cores.

Sharding: no collectives. Core c handles batch c//2, query rows (c%2)*1024..+1024,
all 16 heads. Each core needs full K/V for its batch; W_out/b_out replicated.
The per-core output block [1024, 1024] is the final projected output for those
query rows, so the host just concatenates.

Per-core pipeline (all matmuls bf16, fp32 PSUM accumulation):
  - softmax exp is split across TWO engines: ScalarE exact Exp activation on
    ~20/32 tiles per head pair, DVE on the other 12 via a Schraudolph-style
    int16 trick: bf16_bits(exp(s*SCALE)) ~= int16(s*EXPA + EXPB), one
    tensor_scalar (mult,add) op with an int16-bitcast view of the bf16 expt
    tile.  (DVE fp32->int16 convert truncates; EXPB folds in the +0.5 and the
    Schraudolph centering constant.)
  - psum: 2 score slots [128,1024] + 2 AV accumulators [65,1024] = 8 banks.
    Issue order is software-pipelined: scores(jc+1) is emitted BEFORE AV(jc),
    so the steady-state chain is exp(jc) -> scores(jc+1) (~1.6us) rather than
    exp -> AV -> scores.  Score matmuls are emitted h0,h64 adjacent so the
    row-disjoint pairs execute concurrently in the PE array.
  - AV lhsT = v_aug [128 j, 65] (ones column -> softmax sums for free),
    accumulated over jc with start/stop into av[h2] psum.
  - normalize: DVE copy av->SBUF (releases psum), fast reciprocal of the sums
    row, gpsimd partition_broadcast, DVE multiply -> attT bf16.  The ops are
    spread over the next head pair's jc loop so they never head-of-line block
    the DVE FIFO at a pair boundary.
  - DMA: only SWDGE (gpsimd queue) can cast fp32->bf16, so q/k/v/w casts all
    live there, emitted in need-order (w in 8 column chunks spread over
    hp=2..5; v per-head chunks 2 pairs ahead).  Transposed loads (d on
    partitions) via HWDGE DMA-transpose on the sync queue; k chunk-0/1
    transposes go on the scalar HWDGE queue so the first scores matmul is
    gated by max(q0,k0) instead of their sum.
  - projection: final[i, e] = attT.T @ WT accumulated over the 8 d-chunks in
    psum, bias added on DVE from a partition-broadcast bias tile, fp32 out.
"""

import numpy as np

import concourse.bass as bass
import concourse.tile as tile
from concourse import bacc, mybir
from concourse.bass_utils import run_bass_kernel_spmd

B = 4
S = 2048
DM = 1024
H = 16
DK = 64
SCALE = DK**-0.5
I = 1024  # local query rows per core
NJC = S // 128  # 16 j-chunks
NHP = H // 2  # 8 head pairs == 8 d-chunks of the model dim

F32 = mybir.dt.float32
BF16 = mybir.dt.bfloat16
I16 = mybir.dt.int16

# Schraudolph exp constants: int16(s*EXPA + EXPB) viewed as bf16 bits is
# exp(s*SCALE) to within ~+-3.2%.  EXPA = SCALE*log2(e)*128; EXPB = 16256
# (=127<<7) - 7.5 (spread centering); HW convert rounds-to-nearest (CoreSim
# truncates — sim error is slightly off-center, HW is what matters).
EXPA = SCALE * np.log2(np.e) * 128.0
EXPB = 16248.5

N_WARMUP = 26  # bridge the DMA prelude so HAM never re-throttles


def build(nc: bass.Bass):
    q = nc.dram_tensor("q", [I, DM], F32, kind="ExternalInput").ap()
    k = nc.dram_tensor("k", [S, DM], F32, kind="ExternalInput").ap()
    v = nc.dram_tensor("v", [S, DM], F32, kind="ExternalInput").ap()
    w = nc.dram_tensor("w", [DM, DM], F32, kind="ExternalInput").ap()
    b = nc.dram_tensor("b", [DM], F32, kind="ExternalInput").ap()
    out = nc.dram_tensor("out", [I, DM], F32, kind="ExternalOutput").ap()

    # bf16 DRAM staging for q/k column chunks 1-2 (chunk 0 goes through an
    # on-chip HWDGE-load + DVE-cast + block-transpose path that avoids the
    # SWDGE completion-latency on the critical prelude path).
    CSPLIT = [(0, 128), (128, 512), (512, 1024)]
    q_bf = [None] + [
        nc.dram_tensor(f"q_bf{i}", [I, c1 - c0], BF16).ap()
        for i, (c0, c1) in list(enumerate(CSPLIT))[1:]
    ]
    k_bf = [None] + [
        nc.dram_tensor(f"k_bf{i}", [S, c1 - c0], BF16).ap()
        for i, (c0, c1) in list(enumerate(CSPLIT))[1:]
    ]
    w_bf = nc.dram_tensor("w_bf", [DM, DM], BF16).ap()
    # v bounced through DRAM as bf16 (2 big SWDGE casts), then filled into
    # SBUF per-head by non-cast HWDGE loads on the sync queue — keeps the
    # gpsimd DMA stream down to 9 descriptors so the normalize broadcasts
    # are never starved behind semaphore-recycle fences.
    v_bfd = [
        nc.dram_tensor("v_bfd0", [S, 256], BF16).ap(),
        nc.dram_tensor("v_bfd1", [S, 768], BF16).ap(),
    ]

    with tile.TileContext(nc) as tc:
        with (
            tc.tile_pool(name="persist", bufs=1) as pers,
            tc.tile_pool(name="expp", bufs=4) as expp,
            tc.tile_pool(name="avsbp", bufs=1) as avsbp,
            tc.tile_pool(name="nrmp", bufs=1) as nrmp,
            tc.tile_pool(name="finp", bufs=2) as finp,
        ):
            warm_sb = pers.tile([128, 512], BF16, name="warm_sb", tag="warm_sb")
            nc.vector.memset(warm_sb[:, :], 0.0)

            # ---- v_aug: all 16 heads side by side, ones column at 64 ----
            vA_all = pers.tile(
                [128, H * NJC * 65], BF16, name="vA_all", tag="vA_all"
            )
            vA4 = vA_all[:, :].rearrange("p (h jc e) -> p h jc e", h=H, e=65)
            nc.vector.memset(vA4[:, :, :, DK], 1.0)

            def load_v_h(h):
                # non-cast HWDGE load from the bf16 bounce buffer (sync queue)
                if h < 4:
                    src = v_bfd[0][:, h * DK : (h + 1) * DK]
                else:
                    src = v_bfd[1][:, (h - 4) * DK : (h - 3) * DK]
                nc.sync.dma_start(
                    out=vA4[:, h, :, 0:DK],
                    in_=src.rearrange("(jc p) d -> p jc d", p=128),
                )

            def vA(h):
                return vA_all[:, h * NJC * 65 : (h + 1) * NJC * 65]

            # ---- SWDGE (gpsimd) cast queue: 7 DMAs total, in need-order ----
            nc.gpsimd.dma_start(out=q_bf[1][:, :], in_=q[:, 128:512])
            nc.gpsimd.dma_start(out=k_bf[1][:, :], in_=k[:, 128:512])
            nc.gpsimd.dma_start(out=v_bfd[0][:, :], in_=v[:, 0:256])

            def gp_prefetch(hp):
                if hp == 0:
                    nc.gpsimd.dma_start(out=v_bfd[1][:, :], in_=v[:, 256:1024])
                    nc.gpsimd.dma_start(out=q_bf[2][:, :], in_=q[:, 512:1024])
                    nc.gpsimd.dma_start(out=k_bf[2][:, :], in_=k[:, 512:1024])
                elif hp == 1:
                    nc.gpsimd.dma_start(out=w_bf[:, :], in_=w[:, :])

            # ---- HWDGE transposed loads: qT/kT with head_dim on partitions ----
            qTh, kTh = [], []
            for i, (c0, c1) in enumerate(CSPLIT):
                nb = (c1 - c0) // 128
                qt = pers.tile([128, nb, I], BF16, name=f"qTh{i}", tag=f"qTh{i}")
                qTh.append(qt)
                kt = pers.tile([128, nb, S], BF16, name=f"kTh{i}", tag=f"kTh{i}")
                kTh.append(kt)
            # prelude chunk 0: fp32 HWDGE loads (fast completion), DVE cast,
            # then per-128-block TensorE transposes (keeps the PE busy so HAM
            # stays warm through the prelude; DMA-transposing these 24 small
            # blocks would serialize ~5us each on the ring).
            q0_sb = pers.tile([128, I // 128, 128], F32, name="q0_sb", tag="q0_sb")
            k0_sb = pers.tile([128, S // 128, 128], F32, name="k0_sb", tag="k0_sb")
            nc.sync.dma_start(
                out=q0_sb[:, :, :],
                in_=q[:, 0:128].rearrange("(rb p) c -> p rb c", p=128),
            )
            nc.scalar.dma_start(
                out=k0_sb[:, :, :],
                in_=k[:, 0:128].rearrange("(rb p) c -> p rb c", p=128),
            )
            q0_bfs = pers.tile([128, I // 128, 128], BF16, name="q0_bfs", tag="q0_bfs")
            k0_bfs = pers.tile([128, S // 128, 128], BF16, name="k0_bfs", tag="k0_bfs")
            nc.vector.tensor_copy(q0_bfs[:, :, :], q0_sb[:, :, :])
            nc.vector.tensor_copy(k0_bfs[:, :, :], k0_sb[:, :, :])
            ident = pers.tile([128, 128], BF16, name="ident", tag="ident")
            nc.vector.memset(ident[:, :], 1.0)
            nc.gpsimd.affine_select(
                ident[:, :],
                ident[:, :],
                pattern=[[1, 128]],
                compare_op=mybir.AluOpType.is_equal,
                fill=0.0,
                base=0,
                channel_multiplier=-1,
            )
            nc.sync.dma_start(out=qTh[1][:, :, :], in_=q_bf[1][:, :], transpose=True)
            nc.scalar.dma_start(out=kTh[1][:, :, :], in_=k_bf[1][:, :], transpose=True)
            for h in range(4):
                load_v_h(h)

            wT_all = pers.tile([128, NHP, DM], BF16, name="wT_all", tag="wT_all")
            bias_sb = pers.tile([1, DM], F32, name="bias_sb", tag="bias_sb")
            nc.sync.dma_start(out=bias_sb[:, :], in_=b[None, :])
            bias_bc = pers.tile([128, DM], F32, name="bias_bc", tag="bias_bc")

            def sp_prefetch(hp):
                if 0 <= hp <= 5:
                    load_v_h(2 * hp + 4)
                    load_v_h(2 * hp + 5)
                if hp == 1:
                    nc.sync.dma_start(
                        out=qTh[2][:, :, :], in_=q_bf[2][:, :], transpose=True
                    )
                    nc.sync.dma_start(
                        out=kTh[2][:, :, :], in_=k_bf[2][:, :], transpose=True
                    )
                elif hp == 3:
                    nc.sync.dma_start(
                        out=wT_all[:, :, :], in_=w_bf[:, :], transpose=True
                    )

            def _chunk(hp):
                return (0, 0) if hp == 0 else ((1, hp - 1) if hp < 4 else (2, hp - 4))

            def qT(hp):
                i, e = _chunk(hp)
                return qTh[i][:, e, :]

            def kT(hp):
                i, e = _chunk(hp)
                return kTh[i][:, e, :]

            def wT(dc):
                return wT_all[:, dc, :]

            attT = []
            for hp in range(NHP):
                at = pers.tile([128, I], BF16, name=f"attT{hp}", tag=f"attT{hp}")
                attT.append(at)

            # PE warmup covering the DMA prelude, then chunk-0 PE transposes
            with tc.tile_pool(name="warmp", bufs=1, space="PSUM") as warmp:
                wps = warmp.tile([128, 512], F32, name="wps", tag="wps")
                for _ in range(N_WARMUP):
                    nc.tensor.matmul(
                        wps[:, :],
                        warm_sb[:, 0:128],
                        warm_sb[:, :],
                        start=True,
                        stop=True,
                        skip_group_check=True,
                    )

                ntp = [0]

                def pe_transpose(dst, src_bfs, rb):
                    tp = warmp.tile([128, 128], BF16, name=f"tp{rb}", tag="tp", bufs=3)
                    nc.tensor.transpose(tp[:, :], src_bfs[:, rb, :], ident[:, :])
                    nc.vector.tensor_copy(
                        dst[:, 0, rb * 128 : (rb + 1) * 128], tp[:, :]
                    )
                    # transpose-mode doesn't count as PE-busy for the HAM
                    # clock gate — tick a real matmul every few blocks
                    ntp[0] += 1
                    if ntp[0] % 3 == 0:
                        nc.tensor.matmul(
                            wps[:, :],
                            warm_sb[:, 0:128],
                            warm_sb[:, :],
                            start=True,
                            stop=True,
                            skip_group_check=True,
                        )

                # k block 0 + the q blocks first: scores(jc=0) only needs
                # them (subtile deps), so the first real matmul lands early
                pe_transpose(kTh[0], k0_bfs, 0)
                for rb in range(I // 128):
                    pe_transpose(qTh[0], q0_bfs, rb)
                for rb in range(1, S // 128):
                    pe_transpose(kTh[0], k0_bfs, rb)

            # ---- attention ----
            with (
                tc.tile_pool(name="smmp", bufs=1, space="PSUM") as smmp,
                tc.tile_pool(name="avp", bufs=1, space="PSUM") as avp,
            ):
                # deferred emission state
                pend_av = None  # (expt pair, av pair, jc, hp) awaiting AV matmuls
                norm_q = []  # deferred normalize callbacks, one popped per jc

                def _normalize_ops(p_hp, p_av):
                    asb = []
                    sums = []
                    # immediate: copies that release the av psum slots (h1
                    # first — its slot is needed sooner by the next pair).
                    # asb copies on DVE; sums-row copies on ScalarE (spreads
                    # the release work over both engines).
                    # NOTE: reciprocal_approx_fast (custom DVE op) corrupts
                    # data when its input is partition-shifted, so the sums
                    # row is copied to partition 0 by a plain copy first.
                    for h2 in (1, 0):
                        a = avsbp.tile(
                            [DK, I], F32, name=f"avsb{p_hp}_{h2}", tag=f"avsb{h2}"
                        )
                        sm = nrmp.tile(
                            [1, I], F32, name=f"sm{p_hp}_{h2}", tag=f"sm{h2}"
                        )
                        nc.vector.tensor_copy(a[:, :], p_av[h2][0:DK, :])
                        nc.scalar.copy(sm[:, :], p_av[h2][DK : DK + 1, :])
                        asb.append(a)
                        sums.append(sm)
                    asb.reverse()
                    sums.reverse()
                    rr = [None, None]
                    rb = [None, None]

                    def mk_recip(h2):
                        def _op():
                            rr[h2] = nrmp.tile(
                                [1, I], F32, name=f"rc{p_hp}_{h2}", tag=f"rc{h2}"
                            )
                            nc.vector.reciprocal_approx_fast(
                                rr[h2][:, :], sums[h2][:, :]
                            )
                            rb[h2] = nrmp.tile(
                                [DK, I], F32, name=f"rb{p_hp}_{h2}", tag=f"rb{h2}"
                            )
                            nc.gpsimd.partition_broadcast(
                                rb[h2][:, :], rr[h2][0:1, :]
                            )

                        return _op

                    def mk_mult(h2):
                        def _op():
                            nc.vector.tensor_mul(
                                attT[p_hp][h2 * DK : (h2 + 1) * DK, :],
                                asb[h2][0:DK, :],
                                rb[h2][:, :],
                            )

                        return _op

                    return [mk_recip(0), mk_recip(1), mk_mult(0), mk_mult(1)]

                av_tiles = {}
                for hp in range(NHP):
                    gp_prefetch(hp)
                    sp_prefetch(hp)
                    av = [
                        avp.tile([65, I], F32, name=f"av{hp}_{h2}", tag=f"av{h2}")
                        for h2 in range(2)
                    ]
                    av_tiles[hp] = av
                    for jc in range(NJC):
                        # scores into ONE 4-bank psum tile (h2*1024 + ih*512
                        # columns): both h2 exps then run CONCURRENTLY on the
                        # two engines and the slots free together, so the
                        # next jc's h0/h64 score pairs issue adjacent and
                        # overlap in the PE array (disjoint row groups).
                        smm = smmp.tile(
                            [128, 2 * I], F32, name=f"smm{hp}_{jc}", tag="smm"
                        )
                        for ih in range(2):
                            for h2 in range(2):
                                nc.tensor.matmul(
                                    smm[
                                        :,
                                        h2 * I + ih * 512 : h2 * I + (ih + 1) * 512,
                                    ],
                                    kT(hp)[
                                        h2 * DK : (h2 + 1) * DK,
                                        jc * 128 : (jc + 1) * 128,
                                    ],
                                    qT(hp)[
                                        h2 * DK : (h2 + 1) * DK,
                                        ih * 512 : (ih + 1) * 512,
                                    ],
                                    start=True,
                                    stop=True,
                                    tile_position=(h2 * DK, 0),
                                )
                        # exp: h2=0 on ScalarE (exact), h2=1 on DVE (int16
                        # Schraudolph), concurrently
                        expt = [
                            expp.tile(
                                [128, I], BF16, name=f"ex{hp}_{jc}_{h2}", tag="expt"
                            )
                            for h2 in range(2)
                        ]
                        nc.scalar.activation(
                            expt[0][:, :],
                            smm[:, 0:I],
                            mybir.ActivationFunctionType.Exp,
                            scale=SCALE,
                        )
                        nc.vector.tensor_scalar(
                            expt[1][:, :].bitcast(I16),
                            smm[:, I : 2 * I],
                            EXPA,
                            EXPB,
                            mybir.AluOpType.mult,
                            mybir.AluOpType.add,
                        )
                        # AV of the previous jc (software pipelining)
                        if pend_av is not None:
                            p_expt, p_av, p_jc, p_hp = pend_av
                            for h2 in (1, 0):
                                for ih in range(2):
                                    nc.tensor.matmul(
                                        p_av[h2][:, ih * 512 : (ih + 1) * 512],
                                        vA(2 * p_hp + h2)[
                                            :, p_jc * 65 : p_jc * 65 + 65
                                        ],
                                        p_expt[h2][:, ih * 512 : (ih + 1) * 512],
                                        start=(p_jc == 0),
                                        stop=(p_jc == NJC - 1),
                                        skip_group_check=True,
                                    )
                            if p_jc == NJC - 1:
                                # pair p_hp done: emit psum-releasing copies now
                                # (h1 first: its slot is needed sooner), defer
                                # the rest across upcoming jc's
                                norm_q.extend(_normalize_ops(p_hp, av_tiles[p_hp]))
                        pend_av = (expt, av, jc, hp)
                        # pop at most one deferred normalize op per jc
                        if norm_q:
                            norm_q.pop(0)()

                # flush: last jc's AV + remaining normalize ops
                p_expt, p_av, p_jc, p_hp = pend_av
                for h2 in (1, 0):
                    for ih in range(2):
                        nc.tensor.matmul(
                            p_av[h2][:, ih * 512 : (ih + 1) * 512],
                            vA(2 * p_hp + h2)[:, p_jc * 65 : p_jc * 65 + 65],
                            p_expt[h2][:, ih * 512 : (ih + 1) * 512],
                            start=(p_jc == 0),
                            stop=(p_jc == NJC - 1),
                            skip_group_check=True,
                        )
                norm_q.extend(_normalize_ops(p_hp, av_tiles[p_hp]))
                for op in norm_q:
                    op()
                norm_q = []

                # keep PE warm across the normalize -> projection handoff
                wps2 = smmp.tile([128, I], F32, name="wps2", tag="smm")
                for _ in range(8):
                    nc.tensor.matmul(
                        wps2[:, 0:512],
                        warm_sb[:, 0:128],
                        warm_sb[:, :],
                        start=True,
                        stop=True,
                        skip_group_check=True,
                    )

            # bias broadcast: end of the gpsimd stream, needed only by proj
            nc.gpsimd.partition_broadcast(bias_bc[:, :], bias_sb[0:1, :])

            # ---- output projection ----
            with tc.tile_pool(name="projp", bufs=4, space="PSUM") as projp:
                for ic in range(I // 128):
                    pp = [
                        projp.tile([128, 512], F32, name=f"pp{ic}_{ec}", tag="pp")
                        for ec in range(2)
                    ]
                    for dc in range(NHP):
                        for ec in range(2):
                            nc.tensor.matmul(
                                pp[ec][:, :],
                                attT[dc][:, ic * 128 : (ic + 1) * 128],
                                wT(dc)[:, ec * 512 : (ec + 1) * 512],
                                start=(dc == 0),
                                stop=(dc == NHP - 1),
                                skip_group_check=True,
                            )
                    fin = finp.tile([128, DM], F32, name=f"fin{ic}", tag="fin")
                    for ec in range(2):
                        nc.vector.tensor_add(
                            fin[:, ec * 512 : (ec + 1) * 512],
                            pp[ec][:, :],
                            bias_bc[:, ec * 512 : (ec + 1) * 512],
                        )
                    nc.sync.dma_start(
                        out=out[ic * 128 : (ic + 1) * 128, :], in_=fin[:, :]
                    )
    return nc


_NC_CACHE = {}


def _get_nc():
    if "nc" not in _NC_CACHE:
        nc = bacc.Bacc("TRN2", target_bir_lowering=False, debug=False)
        build(nc)
        nc.compile()
        _NC_CACHE["nc"] = nc
    return _NC_CACHE["nc"]


def kernel(q, k, v, W_out, b_out, _trace=False, _trace_kwargs=None):
    q = np.asarray(q, dtype=np.float32)
    k = np.asarray(k, dtype=np.float32)
    v = np.asarray(v, dtype=np.float32)
    W_out = np.ascontiguousarray(np.asarray(W_out, dtype=np.float32))
    b_out = np.ascontiguousarray(np.asarray(b_out, dtype=np.float32))

    nc = _get_nc()
    in_maps = []
    for c in range(8):
        bi, half = c // 2, c % 2
        in_maps.append(
            {
                "q": np.ascontiguousarray(q[bi, half * I : (half + 1) * I, :]),
                "k": np.ascontiguousarray(k[bi]),
                "v": np.ascontiguousarray(v[bi]),
                "w": W_out,
                "b": b_out,
            }
        )
    res = run_bass_kernel_spmd(
        nc,
        in_maps,
        core_ids=list(range(8)),
        trace=_trace,
        **(_trace_kwargs or {}),
    )
    out = np.empty((B, S, DM), np.float32)
    for c in range(8):
        bi, half = c // 2, c % 2
        out[bi, half * I : (half + 1) * I, :] = res.results[c]["out"]
    if _trace:
        return out, res
    return out


# revision 34
# speedup vs baseline: 1.2391x; 1.0958x over previous
"""Multi-head attention (B=4, S=2048, D=1024, H=16) + output projection on 8 trn2 cores.

Sharding: no collectives. Core c handles batch c//2, query rows (c%2)*1024..+1024,
all 16 heads. Each core needs full K/V for its batch; W_out/b_out replicated.
The per-core output block [1024, 1024] is the final projected output for those
query rows, so the host just concatenates.

Per-core pipeline (all matmuls bf16, fp32 PSUM accumulation):
  - softmax exp is split across TWO engines: ScalarE exact Exp activation on
    ~20/32 tiles per head pair, DVE on the other 12 via a Schraudolph-style
    int16 trick: bf16_bits(exp(s*SCALE)) ~= int16(s*EXPA + EXPB), one
    tensor_scalar (mult,add) op with an int16-bitcast view of the bf16 expt
    tile.  (DVE fp32->int16 convert truncates; EXPB folds in the +0.5 and the
    Schraudolph centering constant.)
  - psum: 2 score slots [128,1024] + 2 AV accumulators [65,1024] = 8 banks.
    Issue order is software-pipelined: scores(jc+1) is emitted BEFORE AV(jc),
    so the steady-state chain is exp(jc) -> scores(jc+1) (~1.6us) rather than
    exp -> AV -> scores.  Score matmuls are emitted h0,h64 adjacent so the
    row-disjoint pairs execute concurrently in the PE array.
  - AV lhsT = v_aug [128 j, 65] (ones column -> softmax sums for free),
    accumulated over jc with start/stop into av[h2] psum.
  - normalize: DVE copy av->SBUF (releases psum), fast reciprocal of the sums
    row, gpsimd partition_broadcast, DVE multiply -> attT bf16.  The ops are
    spread over the next head pair's jc loop so they never head-of-line block
    the DVE FIFO at a pair boundary.
  - DMA: only SWDGE (gpsimd queue) can cast fp32->bf16, so q/k/v/w casts all
    live there, emitted in need-order (w in 8 column chunks spread over
    hp=2..5; v per-head chunks 2 pairs ahead).  Transposed loads (d on
    partitions) via HWDGE DMA-transpose on the sync queue; k chunk-0/1
    transposes go on the scalar HWDGE queue so the first scores matmul is
    gated by max(q0,k0) instead of their sum.
  - projection: final[i, e] = attT.T @ WT accumulated over the 8 d-chunks in
    psum, bias added on DVE from a partition-broadcast bias tile, fp32 out.
"""

import numpy as np

import concourse.bass as bass
import concourse.tile as tile
from concourse import bacc, mybir
from concourse.bass_utils import run_bass_kernel_spmd

B = 4
S = 2048
DM = 1024
H = 16
DK = 64
SCALE = DK**-0.5
I = 1024  # local query rows per core
NJC = S // 128  # 16 j-chunks
NHP = H // 2  # 8 head pairs == 8 d-chunks of the model dim

F32 = mybir.dt.float32
BF16 = mybir.dt.bfloat16
I16 = mybir.dt.int16

# Schraudolph exp constants: int16(s*EXPA + EXPB) viewed as bf16 bits is
# exp(s*SCALE) to within ~+-3.2%.  EXPA = SCALE*log2(e)*128; EXPB = 16256
# (=127<<7) - 7.5 (spread centering); HW convert rounds-to-nearest (CoreSim
# truncates — sim error is slightly off-center, HW is what matters).
EXPA = SCALE * np.log2(np.e) * 128.0
EXPB = 16248.5

N_WARMUP = 26  # bridge the DMA prelude so HAM never re-throttles


def build(nc: bass.Bass):
    q = nc.dram_tensor("q", [I, DM], F32, kind="ExternalInput").ap()
    k = nc.dram_tensor("k", [S, DM], F32, kind="ExternalInput").ap()
    v = nc.dram_tensor("v", [S, DM], F32, kind="ExternalInput").ap()
    w = nc.dram_tensor("w", [DM, DM], F32, kind="ExternalInput").ap()
    b = nc.dram_tensor("b", [DM], F32, kind="ExternalInput").ap()
    out = nc.dram_tensor("out", [I, DM], F32, kind="ExternalOutput").ap()

    # bf16 DRAM staging for q/k column chunks 1-2 (chunk 0 goes through an
    # on-chip HWDGE-load + DVE-cast + block-transpose path that avoids the
    # SWDGE completion-latency on the critical prelude path).
    CSPLIT = [(0, 128), (128, 512), (512, 1024)]
    q_bf = [None] + [
        nc.dram_tensor(f"q_bf{i}", [I, c1 - c0], BF16).ap()
        for i, (c0, c1) in list(enumerate(CSPLIT))[1:]
    ]
    k_bf = [None] + [
        nc.dram_tensor(f"k_bf{i}", [S, c1 - c0], BF16).ap()
        for i, (c0, c1) in list(enumerate(CSPLIT))[1:]
    ]
    w_bf = nc.dram_tensor("w_bf", [DM, DM], BF16).ap()
    # v bounced through DRAM as bf16 (2 big SWDGE casts), then filled into
    # SBUF per-head by non-cast HWDGE loads on the sync queue — keeps the
    # gpsimd DMA stream down to 9 descriptors so the normalize broadcasts
    # are never starved behind semaphore-recycle fences.
    v_bfd = [
        nc.dram_tensor("v_bfd0", [S, 256], BF16).ap(),
        nc.dram_tensor("v_bfd1", [S, 768], BF16).ap(),
    ]

    with tile.TileContext(nc) as tc:
        with (
            tc.tile_pool(name="persist", bufs=1) as pers,
            tc.tile_pool(name="expp", bufs=4) as expp,
            tc.tile_pool(name="avsbp", bufs=1) as avsbp,
            tc.tile_pool(name="nrmp", bufs=1) as nrmp,
            tc.tile_pool(name="finp", bufs=2) as finp,
        ):
            warm_sb = pers.tile([128, 512], BF16, name="warm_sb", tag="warm_sb")
            nc.vector.memset(warm_sb[:, :], 0.0)

            # ---- v_aug: all 16 heads side by side, ones column at 64 ----
            vA_all = pers.tile(
                [128, H * NJC * 65], BF16, name="vA_all", tag="vA_all"
            )
            vA4 = vA_all[:, :].rearrange("p (h jc e) -> p h jc e", h=H, e=65)
            nc.vector.memset(vA4[:, :, :, DK], 1.0)

            def load_v_jc(quad, jc):
                # per-jc all-heads-in-quad loads: contiguous 512B/1.5KB
                # elements (per-HEAD loads have 128B elements and take
                # 9-15us each on the ring)
                if quad == 0:
                    src = v_bfd[0][jc * 128 : (jc + 1) * 128, :]
                    dst = vA4[:, 0:4, jc : jc + 1, 0:DK]
                else:
                    src = v_bfd[1][jc * 128 : (jc + 1) * 128, :]
                    dst = vA4[:, 4:16, jc : jc + 1, 0:DK]
                nc.sync.dma_start(
                    out=dst.rearrange("p h one d -> p (one h) d"),
                    in_=src.rearrange("p (h d) -> p h d", d=DK),
                )

            def vA(h):
                return vA_all[:, h * NJC * 65 : (h + 1) * NJC * 65]

            # ---- SWDGE (gpsimd) cast queue: 7 DMAs total, in need-order ----
            nc.gpsimd.dma_start(out=q_bf[1][:, :], in_=q[:, 128:512])
            nc.gpsimd.dma_start(out=k_bf[1][:, :], in_=k[:, 128:512])
            nc.gpsimd.dma_start(out=v_bfd[0][:, :], in_=v[:, 0:256])

            def gp_prefetch(hp):
                if hp == 0:
                    nc.gpsimd.dma_start(out=v_bfd[1][:, :], in_=v[:, 256:1024])
                    nc.gpsimd.dma_start(out=q_bf[2][:, :], in_=q[:, 512:1024])
                    nc.gpsimd.dma_start(out=k_bf[2][:, :], in_=k[:, 512:1024])
                elif hp == 1:
                    nc.gpsimd.dma_start(out=w_bf[:, :], in_=w[:, :])

            # ---- HWDGE transposed loads: qT/kT with head_dim on partitions ----
            qTh, kTh = [], []
            for i, (c0, c1) in enumerate(CSPLIT):
                nb = (c1 - c0) // 128
                qt = pers.tile([128, nb, I], BF16, name=f"qTh{i}", tag=f"qTh{i}")
                qTh.append(qt)
                kt = pers.tile([128, nb, S], BF16, name=f"kTh{i}", tag=f"kTh{i}")
                kTh.append(kt)
            # prelude chunk 0: fp32 HWDGE loads (fast completion), DVE cast,
            # then per-128-block TensorE transposes (keeps the PE busy so HAM
            # stays warm through the prelude; DMA-transposing these 24 small
            # blocks would serialize ~5us each on the ring).
            q0_sb = pers.tile([128, I // 128, 128], F32, name="q0_sb", tag="q0_sb")
            k0_sb = pers.tile([128, S // 128, 128], F32, name="k0_sb", tag="k0_sb")
            nc.sync.dma_start(
                out=q0_sb[:, :, :],
                in_=q[:, 0:128].rearrange("(rb p) c -> p rb c", p=128),
            )
            nc.scalar.dma_start(
                out=k0_sb[:, :, :],
                in_=k[:, 0:128].rearrange("(rb p) c -> p rb c", p=128),
            )
            q0_bfs = pers.tile([128, I // 128, 128], BF16, name="q0_bfs", tag="q0_bfs")
            k0_bfs = pers.tile([128, S // 128, 128], BF16, name="k0_bfs", tag="k0_bfs")
            nc.vector.tensor_copy(q0_bfs[:, :, :], q0_sb[:, :, :])
            nc.vector.tensor_copy(k0_bfs[:, :, :], k0_sb[:, :, :])
            ident = pers.tile([128, 128], BF16, name="ident", tag="ident")
            nc.vector.memset(ident[:, :], 1.0)
            nc.gpsimd.affine_select(
                ident[:, :],
                ident[:, :],
                pattern=[[1, 128]],
                compare_op=mybir.AluOpType.is_equal,
                fill=0.0,
                base=0,
                channel_multiplier=-1,
            )
            for jc in range(NJC):
                load_v_jc(0, jc)
            nc.sync.dma_start(out=qTh[1][:, :, :], in_=q_bf[1][:, :], transpose=True)
            nc.scalar.dma_start(out=kTh[1][:, :, :], in_=k_bf[1][:, :], transpose=True)

            wT_all = pers.tile([128, NHP, DM], BF16, name="wT_all", tag="wT_all")
            bias_sb = pers.tile([1, DM], F32, name="bias_sb", tag="bias_sb")
            nc.sync.dma_start(out=bias_sb[:, :], in_=b[None, :])
            bias_bc = pers.tile([128, DM], F32, name="bias_bc", tag="bias_bc")

            def sp_prefetch(hp):
                if hp in (0, 1):
                    # heads 4-15, 8 jc-slices per hp (needed from hp2 on)
                    for jc in range(hp * 8, hp * 8 + 8):
                        load_v_jc(1, jc)
                if hp == 1:
                    nc.sync.dma_start(
                        out=qTh[2][:, :, :], in_=q_bf[2][:, :], transpose=True
                    )
                    nc.sync.dma_start(
                        out=kTh[2][:, :, :], in_=k_bf[2][:, :], transpose=True
                    )
                elif hp == 3:
                    nc.sync.dma_start(
                        out=wT_all[:, :, :], in_=w_bf[:, :], transpose=True
                    )

            def _chunk(hp):
                return (0, 0) if hp == 0 else ((1, hp - 1) if hp < 4 else (2, hp - 4))

            def qT(hp):
                i, e = _chunk(hp)
                return qTh[i][:, e, :]

            def kT(hp):
                i, e = _chunk(hp)
                return kTh[i][:, e, :]

            def wT(dc):
                return wT_all[:, dc, :]

            attT = []
            for hp in range(NHP):
                at = pers.tile([128, I], BF16, name=f"attT{hp}", tag=f"attT{hp}")
                attT.append(at)

            # PE warmup covering the DMA prelude, then chunk-0 PE transposes
            with tc.tile_pool(name="warmp", bufs=1, space="PSUM") as warmp:
                wps = warmp.tile([128, 512], F32, name="wps", tag="wps")
                for _ in range(N_WARMUP):
                    nc.tensor.matmul(
                        wps[:, :],
                        warm_sb[:, 0:128],
                        warm_sb[:, :],
                        start=True,
                        stop=True,
                        skip_group_check=True,
                    )

                ntp = [0]

                def pe_transpose(dst, src_bfs, rb):
                    tp = warmp.tile([128, 128], BF16, name=f"tp{rb}", tag="tp", bufs=3)
                    nc.tensor.transpose(tp[:, :], src_bfs[:, rb, :], ident[:, :])
                    nc.vector.tensor_copy(
                        dst[:, 0, rb * 128 : (rb + 1) * 128], tp[:, :]
                    )
                    # transpose-mode doesn't count as PE-busy for the HAM
                    # clock gate — tick a real matmul every few blocks
                    ntp[0] += 1
                    if ntp[0] % 3 == 0:
                        nc.tensor.matmul(
                            wps[:, :],
                            warm_sb[:, 0:128],
                            warm_sb[:, :],
                            start=True,
                            stop=True,
                            skip_group_check=True,
                        )

                # k block 0 + the q blocks first: scores(jc=0) only needs
                # them (subtile deps), so the first real matmul lands early
                pe_transpose(kTh[0], k0_bfs, 0)
                for rb in range(I // 128):
                    pe_transpose(qTh[0], q0_bfs, rb)
                for rb in range(1, S // 128):
                    pe_transpose(kTh[0], k0_bfs, rb)

            # ---- attention ----
            with (
                tc.tile_pool(name="smmp", bufs=2, space="PSUM") as smmp,
                tc.tile_pool(name="avp", bufs=1, space="PSUM") as avp,
            ):
                # deferred emission state
                pend_av = None  # (expt pair, av pair, jc, hp) awaiting AV matmuls
                norm_q = []  # deferred normalize callbacks, one popped per jc

                def _normalize_ops(p_hp, p_av):
                    asb = []
                    sums = []
                    # immediate: copies that release the av psum slots (h1
                    # first — its slot is needed sooner by the next pair).
                    # asb copies on DVE; sums-row copies on ScalarE (spreads
                    # the release work over both engines).
                    # NOTE: reciprocal_approx_fast (custom DVE op) corrupts
                    # data when its input is partition-shifted, so the sums
                    # row is copied to partition 0 by a plain copy first.
                    for h2 in (1, 0):
                        a = avsbp.tile(
                            [DK, I], F32, name=f"avsb{p_hp}_{h2}", tag=f"avsb{h2}"
                        )
                        sm = nrmp.tile(
                            [1, I], F32, name=f"sm{p_hp}_{h2}", tag=f"sm{h2}"
                        )
                        nc.vector.tensor_copy(a[:, :], p_av[h2][0:DK, :])
                        nc.scalar.copy(sm[:, :], p_av[h2][DK : DK + 1, :])
                        asb.append(a)
                        sums.append(sm)
                    asb.reverse()
                    sums.reverse()
                    rr = [None, None]
                    rb = [None, None]

                    def mk_recip(h2):
                        def _op():
                            rr[h2] = nrmp.tile(
                                [1, I], F32, name=f"rc{p_hp}_{h2}", tag=f"rc{h2}"
                            )
                            nc.vector.reciprocal_approx_fast(
                                rr[h2][:, :], sums[h2][:, :]
                            )
                            rb[h2] = nrmp.tile(
                                [DK, I], F32, name=f"rb{p_hp}_{h2}", tag=f"rb{h2}"
                            )
                            nc.gpsimd.partition_broadcast(
                                rb[h2][:, :], rr[h2][0:1, :]
                            )

                        return _op

                    def mk_mult(h2):
                        def _op():
                            nc.vector.tensor_mul(
                                attT[p_hp][h2 * DK : (h2 + 1) * DK, :],
                                asb[h2][0:DK, :],
                                rb[h2][:, :],
                            )

                        return _op

                    return [mk_recip(0), mk_recip(1), mk_mult(0), mk_mult(1)]

                av_tiles = {}
                for hp in range(NHP):
                    gp_prefetch(hp)
                    sp_prefetch(hp)
                    av = [
                        avp.tile([65, I], F32, name=f"av{hp}_{h2}", tag=f"av{h2}")
                        for h2 in range(2)
                    ]
                    av_tiles[hp] = av
                    for jc in range(NJC):
                        # two 2-bank score tiles (separate tiles: Tile
                        # serializes cross-engine readers of a shared tile).
                        # The h2=1 (DVE-exp-gated, slower) pair is issued
                        # FIRST: the PE's strict FIFO then aligns both slots'
                        # gates each jc, so the h64/h0 pairs (disjoint row
                        # groups) execute concurrently in the PE array.
                        smm = [
                            smmp.tile(
                                [128, I], F32, name=f"smm{hp}_{jc}_{h2}", tag="smm"
                            )
                            for h2 in range(2)
                        ]
                        for ih in range(2):
                            for h2 in (1, 0):
                                nc.tensor.matmul(
                                    smm[h2][:, ih * 512 : (ih + 1) * 512],
                                    kT(hp)[
                                        h2 * DK : (h2 + 1) * DK,
                                        jc * 128 : (jc + 1) * 128,
                                    ],
                                    qT(hp)[
                                        h2 * DK : (h2 + 1) * DK,
                                        ih * 512 : (ih + 1) * 512,
                                    ],
                                    start=True,
                                    stop=True,
                                    tile_position=(h2 * DK, 0),
                                )
                        # exp: h2=0 on ScalarE (exact), h2=1 on DVE (int16
                        # Schraudolph), concurrently
                        expt = [
                            expp.tile(
                                [128, I], BF16, name=f"ex{hp}_{jc}_{h2}", tag="expt"
                            )
                            for h2 in range(2)
                        ]
                        nc.vector.tensor_scalar(
                            expt[1][:, :].bitcast(I16),
                            smm[1][:, :],
                            EXPA,
                            EXPB,
                            mybir.AluOpType.mult,
                            mybir.AluOpType.add,
                        )
                        nc.scalar.activation(
                            expt[0][:, :],
                            smm[0][:, :],
                            mybir.ActivationFunctionType.Exp,
                            scale=SCALE,
                        )
                        # AV of the previous jc (software pipelining)
                        if pend_av is not None:
                            p_expt, p_av, p_jc, p_hp = pend_av
                            for h2 in (1, 0):
                                for ih in range(2):
                                    nc.tensor.matmul(
                                        p_av[h2][:, ih * 512 : (ih + 1) * 512],
                                        vA(2 * p_hp + h2)[
                                            :, p_jc * 65 : p_jc * 65 + 65
                                        ],
                                        p_expt[h2][:, ih * 512 : (ih + 1) * 512],
                                        start=(p_jc == 0),
                                        stop=(p_jc == NJC - 1),
                                        skip_group_check=True,
                                    )
                            if p_jc == NJC - 1:
                                # pair p_hp done: emit psum-releasing copies now
                                # (h1 first: its slot is needed sooner), defer
                                # the rest across upcoming jc's
                                norm_q.extend(_normalize_ops(p_hp, av_tiles[p_hp]))
                        pend_av = (expt, av, jc, hp)
                        # pop at most one deferred normalize op per jc
                        if norm_q:
                            norm_q.pop(0)()

                # flush: last jc's AV + remaining normalize ops
                p_expt, p_av, p_jc, p_hp = pend_av
                for h2 in (1, 0):
                    for ih in range(2):
                        nc.tensor.matmul(
                            p_av[h2][:, ih * 512 : (ih + 1) * 512],
                            vA(2 * p_hp + h2)[:, p_jc * 65 : p_jc * 65 + 65],
                            p_expt[h2][:, ih * 512 : (ih + 1) * 512],
                            start=(p_jc == 0),
                            stop=(p_jc == NJC - 1),
                            skip_group_check=True,
                        )
                norm_q.extend(_normalize_ops(p_hp, av_tiles[p_hp]))
                for op in norm_q:
                    op()
                norm_q = []

                # keep PE warm across the normalize -> projection handoff
                wps2 = smmp.tile([128, I], F32, name="wps2", tag="smm")
                for _ in range(8):
                    nc.tensor.matmul(
                        wps2[:, 0:512],
                        warm_sb[:, 0:128],
                        warm_sb[:, :],
                        start=True,
                        stop=True,
                        skip_group_check=True,
                    )

            # bias broadcast: end of the gpsimd stream, needed only by proj
            nc.gpsimd.partition_broadcast(bias_bc[:, :], bias_sb[0:1, :])

            # ---- output projection ----
            with tc.tile_pool(name="projp", bufs=4, space="PSUM") as projp:
                for ic in range(I // 128):
                    pp = [
                        projp.tile([128, 512], F32, name=f"pp{ic}_{ec}", tag="pp")
                        for ec in range(2)
                    ]
                    for dc in range(NHP):
                        for ec in range(2):
                            nc.tensor.matmul(
                                pp[ec][:, :],
                                attT[dc][:, ic * 128 : (ic + 1) * 128],
                                wT(dc)[:, ec * 512 : (ec + 1) * 512],
                                start=(dc == 0),
                                stop=(dc == NHP - 1),
                                skip_group_check=True,
                            )
                    fin = finp.tile([128, DM], F32, name=f"fin{ic}", tag="fin")
                    for ec in range(2):
                        nc.vector.tensor_add(
                            fin[:, ec * 512 : (ec + 1) * 512],
                            pp[ec][:, :],
                            bias_bc[:, ec * 512 : (ec + 1) * 512],
                        )
                    nc.sync.dma_start(
                        out=out[ic * 128 : (ic + 1) * 128, :], in_=fin[:, :]
                    )
    return nc


_NC_CACHE = {}


def _get_nc():
    if "nc" not in _NC_CACHE:
        nc = bacc.Bacc("TRN2", target_bir_lowering=False, debug=False)
        build(nc)
        nc.compile()
        _NC_CACHE["nc"] = nc
    return _NC_CACHE["nc"]


def kernel(q, k, v, W_out, b_out, _trace=False, _trace_kwargs=None):
    q = np.asarray(q, dtype=np.float32)
    k = np.asarray(k, dtype=np.float32)
    v = np.asarray(v, dtype=np.float32)
    W_out = np.ascontiguousarray(np.asarray(W_out, dtype=np.float32))
    b_out = np.ascontiguousarray(np.asarray(b_out, dtype=np.float32))

    nc = _get_nc()
    in_maps = []
    for c in range(8):
        bi, half = c // 2, c % 2
        in_maps.append(
            {
                "q": np.ascontiguousarray(q[bi, half * I : (half + 1) * I, :]),
                "k": np.ascontiguousarray(k[bi]),
                "v": np.ascontiguousarray(v[bi]),
                "w": W_out,
                "b": b_out,
            }
        )
    res = run_bass_kernel_spmd(
        nc,
        in_maps,
        core_ids=list(range(8)),
        trace=_trace,
        **(_trace_kwargs or {}),
    )
    out = np.empty((B, S, DM), np.float32)
    for c in range(8):
        bi, half = c // 2, c % 2
        out[bi, half * I : (half + 1) * I, :] = res.results[c]["out"]
    if _trace:
        return out, res
    return out
